# revision 1
# baseline (speedup 1.0000x reference)
"""Trainium2 Bass kernel for nn_Bernprop2 (BernNet-style GNN propagation).

Destination-node sharding across 8 cores. Each SpMM stage: dma_gather
source rows (int16 indices into 2-rank chunks, 256B fp16 rows) -> fused
one-hot S build on DVE (single tensor_scalar: (iota==ld)*wv, fp16, 2x
perf mode) -> TensorE fp16 matmul segment-sum accumulating per 128-row
window in PSUM (fp32) -> ACT copy into an fp16 SBUF accumulator.
Inter-stage tables are exchanged with ncfw AllGather (fp16) into
internal Shared DRAM.

Stage graph (5 SpMMs instead of reference's 6):
  s1: sp1 = Anorm @ x         (gather xtab)        -> lx = x - sp1
  s2: sp2 = Anorm @ lx        (gather t_lx)        -> out = c0 x + c1 lx + c2 (lx - sp2)
  s3: z1 = NB @ out           (gather t_out)       -> acc left halves
  s5: v1 = NS @ out           (gather t_out)       -> acc right halves
  s46: [z_pos | z_neg] = NB @ [z1 | v1]  (gather t_zv, dual-width rows)

Tables live in a permuted "device layout": node n -> slot
k*RP + p*W + w  (k=n//R, r=n%R, w=r//P, p=r%P) so every table write is one
contiguous DMA and gather indices within a 2-rank chunk fit in int16.
Table rows are 128 fp16 elems (256B - the dma_gather minimum); single-width
stages use the lower 64, the dual stage uses all 128.
"""

import sys

if "/opt/trn_rl_repo" not in sys.path:
    sys.path.insert(0, "/opt/trn_rl_repo")

import numpy as np

P = 128   # partitions / window rows / tile edges
GCAP = 2048   # max indices per dma_gather call; finer beats coarser on
              # HW (queue interleave + earlier first matmul per cell),
              # but 1280 and 1024 hang the device intermittently —
              # 2048 is the fastest config with zero observed hangs.
SCRATCH = 32768   # SWDGE ring bytes (2048 descriptors)
SINGLE_PACKET = False  # dma_gather packetization mode
BUFS = 5      # buffer depth for blob/V/S pools (5 > 4 > 3 > 2 on HW)
PP_BUFS = 8   # PSUM tile pool depth (window accumulators)


class Cfg:
    def __init__(self, N=100000, E=1250000, D=64, C=8, block_w=8,
                 n_queues=4):
        self.N, self.E, self.D, self.C = N, E, D, C
        self.NQ = n_queues
        assert N % C == 0
        self.R = N // C                     # rows per core
        self.W = -(-self.R // P)            # windows per core
        self.RP = self.W * P                # padded rows per core
        self.NP = self.C * self.RP          # padded table rows
        self.CHUNK = 2 * self.RP            # rows per gather chunk (2 ranks)
        assert self.CHUNK <= 32767
        self.NCH = self.C // 2              # number of chunks
        self.BLOCK_W = block_w              # windows per block
        self.NBLK = -(-self.W // self.BLOCK_W)


def _slot(cfg, n):
    """Global device-table slot for node id array n."""
    k = n // cfg.R
    r = n - k * cfg.R
    return k * cfg.RP + (r % P) * cfg.W + (r // P)


def _chunk_idx(cfg, n):
    """(chunk id, int16 index within chunk) for source node array n."""
    k = n // cfg.R
    r = n - k * cfg.R
    return (k >> 1), (k & 1) * cfg.RP + (r % P) * cfg.W + (r // P)


def _from_dev_rows(cfg, a):
    """[P, W*D] per-core device rows -> [R, D]."""
    full = a.reshape(P, cfg.W, cfg.D).transpose(1, 0, 2).reshape(cfg.RP,
                                                                 cfg.D)
    return full[: cfg.R]


class ShufPack:
    """Per-core padded chunk-sorted layout for the [out | out[shuf]] table.

    Core k's section (RP4 rows): its RP slots permuted so slots whose
    shuf-target lives in chunk c form the contiguous segment
    [O_c, O_c + cnt_kc), padded (idx 0) up to the uniform S_c.
    """

    def __init__(self, cfg, shuf):
        C, R, W, RP, CHUNK = cfg.C, cfg.R, cfg.W, cfg.RP, cfg.CHUNK
        s_all = np.arange(RP)
        p, w = s_all // W, s_all % W
        r = w * P + p                    # local row for each slot
        real = r < R
        per_core = []
        cnts = np.zeros((C, cfg.NCH), np.int64)
        shuf = np.asarray(shuf, np.int64)
        for k in range(C):
            n = np.minimum(k * R + r, cfg.N - 1)
            st = np.where(real, _slot(cfg, shuf[n]), 0)
            c = st // CHUNK
            order = np.argsort(c, kind="stable")
            per_core.append((order, st[order], c[order]))
            cnts[k] = np.bincount(c, minlength=cfg.NCH)
        S_c = ((cnts.max(axis=0) + 127) // 128) * 128
        self.O_c = np.concatenate([[0], np.cumsum(S_c)]).astype(np.int64)
        self.S_c = S_c
        self.RP4 = int(self.O_c[-1])
        self.CHUNK4 = 2 * self.RP4
        assert self.CHUNK4 <= 32767
        self.posmap = np.zeros((C, RP), np.int64)
        self.idxL, self.idxR = [], []
        for k in range(C):
            order, st_o, c_o = per_core[k]
            iL = np.zeros(self.RP4, np.int16)
            iR = np.zeros(self.RP4, np.int16)
            for c in range(cfg.NCH):
                seg = c_o == c
                cnt = int(seg.sum())
                sl = order[seg]
                o = self.O_c[c]
                iL[o:o + cnt] = sl
                iR[o:o + cnt] = st_o[seg] - c * CHUNK
                self.posmap[k, sl] = o + np.arange(cnt)
            self.idxL.append(iL.reshape(-1, 16).T.copy())
            self.idxR.append(iR.reshape(-1, 16).T.copy())

    def gi_fn(self, cfg, col):
        """(chunk, int16 idx) of col nodes in the padded t_oo layout."""
        st = _slot(cfg, col)
        k = st // cfg.RP
        pos = self.posmap[k, st % cfg.RP]
        return (k >> 1), (k & 1) * self.RP4 + pos


class Graph:
    """Shared schedule + per-core blobs for one edge list."""

    def __init__(self, cfg, row, col, wv, gi_fn=None):
        C, R, W, NCH, BW = cfg.C, cfg.R, cfg.W, cfg.NCH, cfg.BLOCK_W
        per_core = []
        counts = np.zeros((C, NCH, W), np.int64)
        for k in range(C):
            m = (row >= k * R) & (row < (k + 1) * R)
            r = row[m] - k * R
            cc, gi = (gi_fn or _chunk_idx)(cfg, col[m])
            win, ld = r // P, r % P
            order = np.lexsort((gi, ld, win, cc, win // BW))
            per_core.append((cc[order], win[order], ld[order], gi[order],
                             wv[m][order]))
            np.add.at(counts[k], (cc[order], win[order]), 1)
        maxc = counts.max(axis=0)                      # [NCH, W]
        ntile = -(-maxc // P)                          # tiles per (c, w) cell
        ntile[0] = np.maximum(ntile[0], 1)             # c0 owns start=True

        # Schedule: blocks -> cells (c, list of (w, ntiles)) in stream order.
        self.blocks = []
        tot = 0
        for b in range(cfg.NBLK):
            ws = range(b * BW, min((b + 1) * BW, W))
            cells = []
            for c in range(NCH):
                wt = [(w, int(ntile[c, w])) for w in ws if ntile[c, w] > 0]
                n = sum(t for _, t in wt) * P
                cells.append((c, tot, n, wt))
                tot += n
            self.blocks.append((list(ws), cells))
        self.total = tot
        self.max_cell = max((n for (_, cells) in self.blocks
                             for (_, _, n, _) in cells), default=0)
        self.max_blk = max((sum(n for (_, _, n, _) in cells)
                            for _, cells in self.blocks), default=0)

        # Per-core blobs in schedule layout.  Pad slots: gi=0 (gathers row 0,
        # harmless), wv=0 (zero weight), ld=0.
        self.gidx, self.ldw, self.wvv = [], [], []
        for k in range(C):
            cc, win, ld, gi, wv_ = per_core[k]
            g16 = np.zeros(tot, np.int16)
            ldf = np.zeros(tot, np.float32)
            wvf = np.zeros(tot, np.float32)
            starts = {}
            for ws_, cells in self.blocks:
                for (c, off, n, wt) in cells:
                    o = off
                    for (w, t) in wt:
                        starts[(c, w)] = o
                        o += t * P
            keys = cc * W + win
            uk, first, cnt = np.unique(keys, return_index=True,
                                       return_counts=True)
            for u, f, n_ in zip(uk, first, cnt):
                c, w = int(u) // W, int(u) % W
                o = starts[(c, w)]
                g16[o:o + n_] = gi[f:f + n_]
                ldf[o:o + n_] = ld[f:f + n_]
                wvf[o:o + n_] = wv_[f:f + n_]
            self.gidx.append(g16.reshape(-1, 16).T.copy())   # [16, tot/16]
            # 4x-expanded fp16 blobs: keeps DVE 2x mode (packed last dim)
            self.ldw.append(np.repeat(ldf.reshape(-1, P).T, 4, axis=1)
                            .astype(np.float16))             # [P, tot/P*4]
            self.wvv.append(np.repeat(wvf.reshape(-1, P).T, 4, axis=1)
                            .astype(np.float16))


# ---------------------------------------------------------------- builder --

def build_program(cfg, graphs, pack, repeat=1, variant="full"):
    """graphs = dict(L=Graph, NB=Graph, NBoo=Graph). Returns compiled nc."""
    import concourse.bacc as bacc
    import concourse.mybir as mybir
    import concourse.tile as tile

    D, W, NP, CHUNK = cfg.D, cfg.W, cfg.NP, cfg.CHUNK
    RP4, CHUNK4 = pack.RP4, pack.CHUNK4
    D2 = 2 * D
    f16 = mybir.dt.float16
    f32 = mybir.dt.float32
    ts = mybir.AluOpType
    nc = bacc.Bacc("TRN2", target_bir_lowering=False, debug=False,
                   num_devices=cfg.C, num_swdge_queues=cfg.NQ,
                   dynamic_dma_scratch_size=SCRATCH)

    # I/O ------------------------------------------------------------------
    xtab = nc.dram_tensor("xtab", [NP, D2], f16, kind="ExternalInput")
    xrows = nc.dram_tensor("xrows", [P, W * D], f16, kind="ExternalInput")
    cob_in = nc.dram_tensor("cob", [P, 4], f32, kind="ExternalInput")
    iota_in = nc.dram_tensor("iota", [P, P], f16, kind="ExternalInput")
    blobs = {}
    for name, g in graphs.items():
        blobs[name] = dict(
            gi=nc.dram_tensor(f"gi_{name}", [16, g.total // 16],
                              mybir.dt.int16, kind="ExternalInput"),
            ld=nc.dram_tensor(f"ld_{name}", [P, g.total // P * 4], f16,
                              kind="ExternalInput"),
            wv=nc.dram_tensor(f"wv_{name}", [P, g.total // P * 4], f16,
                              kind="ExternalInput"),
        )
    giL_in = nc.dram_tensor("giL", [16, RP4 // 16], mybir.dt.int16,
                            kind="ExternalInput")
    giR_in = nc.dram_tensor("giR", [16, RP4 // 16], mybir.dt.int16,
                            kind="ExternalInput")
    out_dev = nc.dram_tensor("out_dev", [P, W * D], f16,
                             kind="ExternalOutput")
    zpos_dev = nc.dram_tensor("zpos_dev", [P, W * D], f16,
                              kind="ExternalOutput")
    zneg_dev = nc.dram_tensor("zneg_dev", [P, W * D], f16,
                              kind="ExternalOutput")

    rg = [list(range(cfg.C))]
    mx = max(max(g.max_cell for g in graphs.values()),
             int(pack.S_c.max()))
    mxb = max(g.max_blk for g in graphs.values())

    with tile.TileContext(nc) as tc:
        with (
            tc.tile_pool(name="const", bufs=1) as constp,
            tc.tile_pool(name="acc", bufs=1) as accp,
            tc.tile_pool(name="blob", bufs=BUFS) as blobp,
            tc.tile_pool(name="vg", bufs=BUFS) as vp,
            tc.tile_pool(name="sm", bufs=BUFS) as sp,
            tc.tile_pool(name="ps", bufs=max(PP_BUFS, cfg.BLOCK_W),
                         space="PSUM") as pp,
            tc.tile_pool(name="dram", bufs=1, space="DRAM") as dp,
        ):
            iota_t = constp.tile([P, P], f16, name="iota_t")
            nc.sync.dma_start(iota_t[:], iota_in[:])
            xr = constp.tile([P, W * D], f16, name="xr")
            nc.sync.dma_start(xr[:], xrows[:])
            lxr = constp.tile([P, W * D], f16, name="lxr")
            outr = constp.tile([P, W * D], f16, name="outr")
            cob = constp.tile([P, 4], f32, name="cob")
            nc.sync.dma_start(cob[:], cob_in[:])
            gtL = constp.tile([P, RP4 // 16], mybir.dt.int16, name="gtL")
            gtR = constp.tile([P, RP4 // 16], mybir.dt.int16, name="gtR")
            for r in range(8):
                nc.sync.dma_start(gtL[16 * r:16 * (r + 1), :], giL_in[:])
                nc.sync.dma_start(gtR[16 * r:16 * (r + 1), :], giR_in[:])
            # wide accumulator: per-window 128-elem rows
            acc = accp.tile([P, W * D2], f16, name="acc_t")
            a3 = acc[:].rearrange("p (w d) -> p w d", d=D2)
            acc_lo = a3[:, :, 0:D]           # [P, W, D] strided views
            acc_hi = a3[:, :, D:D2]
            xr3 = xr[:].rearrange("p (w d) -> p w d", d=D)
            lxr3 = lxr[:].rearrange("p (w d) -> p w d", d=D)
            outr3 = outr[:].rearrange("p (w d) -> p w d", d=D)

            qcnt = [0]

            def spmm(g, blob, table_ap, dual=False, half=0,
                     chunk_rows=CHUNK):
                """One SpMM stage.

                dual: V/psum are 128 wide, result -> full acc rows.
                else: 64 wide, result -> acc half `half` (0=lo, 1=hi).
                """
                wdt = D2 if dual else D
                for ws, cells in g.blocks:
                    blk_n = sum(n for (_, _, n, _) in cells)
                    if blk_n == 0:
                        continue
                    b_off = cells[0][1]
                    gt = blobp.tile([P, mxb // 16], mybir.dt.int16, tag="gt")
                    lt = blobp.tile([P, mxb // P * 4], f16, tag="lt")
                    wt_ = blobp.tile([P, mxb // P * 4], f16, tag="wt")
                    for r in range(8):
                        nc.sync.dma_start(
                            gt[16 * r:16 * (r + 1), : blk_n // 16],
                            blob["gi"][:, b_off // 16:(b_off + blk_n) // 16])
                    nc.sync.dma_start(
                        lt[:, : blk_n // P * 4],
                        blob["ld"][:, b_off // P * 4:(b_off + blk_n) // P * 4])
                    nc.sync.dma_start(
                        wt_[:, : blk_n // P * 4],
                        blob["wv"][:, b_off // P * 4:(b_off + blk_n) // P * 4])
                    ptiles = {w: pp.tile([P, wdt], f32, tag="psw",
                                         name=f"ps_{w}") for w in ws}
                    touched = set()
                    for (c, off, n, wtl) in cells:
                        if n == 0:
                            continue
                        nt = n // P
                        lo = off - b_off
                        V = vp.tile([P, mx // P, D2], f16, tag="V")
                        for g0 in range(0, n, GCAP):
                            gn = min(GCAP, n - g0)
                            nc.gpsimd.dma_gather(
                                V[:, g0 // P:(g0 + gn) // P, :],
                                table_ap[c * chunk_rows:
                                         (c + 1) * chunk_rows, :],
                                gt[:, (lo + g0) // 16:(lo + g0 + gn) // 16],
                                gn, gn, D2, queue_num=qcnt[0] % cfg.NQ,
                                single_packet=SINGLE_PACKET)
                            qcnt[0] += 1
                        S = sp.tile([P, (mx // P) * P], f16, tag="S")
                        s4 = S[:].rearrange("p (t a b) -> p t a b",
                                            a=32, b=4)[:, :nt]
                        iob = iota_t[:].rearrange("p (a b) -> p a b", b=4) \
                            .unsqueeze(1).to_broadcast([P, nt, 32, 4])
                        c4 = lo // P * 4
                        ld4 = lt[:, c4:c4 + nt * 4] \
                            .rearrange("p (t b) -> p t b", b=4) \
                            .unsqueeze(2).to_broadcast([P, nt, 32, 4])
                        nc.vector.tensor_tensor(s4, iob, ld4,
                                                op=ts.is_equal)
                        wv4 = wt_[:, c4:c4 + nt * 4] \
                            .rearrange("p (t b) -> p t b", b=4)
                        if dual:
                            wb = wv4.unsqueeze(2).to_broadcast([P, nt, 32, 4])
                            nc.vector.tensor_tensor(s4, s4, wb, op=ts.mult)
                        else:
                            v4 = V[:, :nt, 0:D].rearrange(
                                "p t (a b) -> p t a b", b=4)
                            wb = wv4.unsqueeze(2).to_broadcast([P, nt, 16, 4])
                            nc.vector.tensor_tensor(v4, v4, wb, op=ts.mult)
                        ti = 0
                        for (w, t) in wtl:
                            for j in range(t):
                                last = (c == max(
                                    cx for (cx, _, nx, wl) in cells
                                    if nx and any(wx == w for wx, _ in wl))
                                    and j == t - 1)
                                nc.tensor.matmul(
                                    ptiles[w][:],
                                    S[:, (ti + j) * P:(ti + j + 1) * P],
                                    V[:, ti + j, 0:wdt],
                                    start=(w not in touched),
                                    stop=last)
                                touched.add(w)
                            ti += t
                    for w in ws:
                        if dual:
                            dst = a3[:, w, :]
                        else:
                            dst = a3[:, w, half * D:half * D + D]
                        nc.any.tensor_copy(dst, ptiles[w][:])

            do_s1 = variant != "empty"
            do_rest = variant in ("noag", "full")
            do_ag = variant == "full"
            for _rep in range(repeat):
                # upper halves of acc rows must be zero for s1/s2 bounces
                nc.vector.memset(acc_hi, 0.0)
                if not do_s1:
                    nc.vector.memset(acc_lo, 0.0)
                if not do_rest:
                    nc.vector.memset(outr[:], 0.0)

                # stage 1: sp1 = Anorm @ x ; lx = x - sp1
                bounce1 = dp.tile([cfg.RP, D2], f16, name="bn_lx")
                t_lx = dp.tile([NP, D2], f16, addr_space="Shared",
                               name="tb_lx")
                if do_s1:
                    with nc.named_scope("s1_L"):
                        spmm(graphs["L"], blobs["L"], xtab[:])
                        nc.vector.tensor_tensor(acc_lo, xr3, acc_lo,
                                                op=ts.subtract)
                        nc.vector.tensor_copy(lxr3, acc_lo)
                    nc.sync.dma_start(
                        bounce1[:].rearrange("(p w) d -> p (w d)", p=P),
                        acc[:])
                if do_ag:
                    with nc.named_scope("ag_lx"):
                        nc.gpsimd.collective_compute(
                            "AllGather", ts.bypass, replica_groups=rg,
                            ins=[bounce1[:].opt()], outs=[t_lx[:].opt()])

                # stage 2: sp2 = Anorm @ lx ; out = c0 x + c1 lx + c2 LLx
                bounce2 = dp.tile([cfg.RP, D2], f16, name="bn_out")
                t_out = dp.tile([NP, D2], f16, addr_space="Shared",
                                name="tb_out")
                if do_rest:
                    with nc.named_scope("s2_L"):
                        spmm(graphs["L"], blobs["L"], t_lx[:])
                        # acc_lo = sp2 -> LLx = lx - sp2
                        nc.vector.tensor_tensor(acc_lo, lxr3, acc_lo,
                                                op=ts.subtract)
                        nc.vector.tensor_scalar(acc_lo, acc_lo, cob[:, 2:3],
                                                None, op0=ts.mult)
                        nc.vector.tensor_scalar(outr[:], xr[:], cob[:, 0:1],
                                                None, op0=ts.mult)
                        nc.vector.tensor_tensor(outr3, outr3, acc_lo,
                                                op=ts.add)
                        nc.vector.tensor_scalar(lxr[:], lxr[:], cob[:, 1:2],
                                                None, op0=ts.mult)
                        nc.vector.tensor_tensor(outr[:], outr[:], lxr[:],
                                                op=ts.add)
                        nc.vector.tensor_copy(acc_lo, outr3)
                nc.sync.dma_start(out_dev[:], outr[:])
                if do_rest:
                    nc.sync.dma_start(
                        bounce2[:].rearrange("(p w) d -> p (w d)", p=P),
                        acc[:])
                if do_ag:
                    with nc.named_scope("ag_out"):
                        nc.gpsimd.collective_compute(
                            "AllGather", ts.bypass, replica_groups=rg,
                            ins=[bounce2[:].opt()], outs=[t_out[:].opt()])

                # build t_oo: rows [out_i | out_shuf_i] in the padded
                # chunk-sorted per-core layout (pack).  Left halves gather
                # from the LOCAL bounce2 (no AG dependency); right halves
                # from t_out after the AG.
                bounce_oo = dp.tile([RP4, D2], f16, name="bn_oo")
                t_oo = dp.tile([cfg.C * RP4, D2], f16, addr_space="Shared",
                               name="tb_oo")
                boo3 = bounce_oo[:].rearrange("(t p) d -> p t d", p=P)
                if do_rest:
                    with nc.named_scope("mk_oo"):
                        for c in range(cfg.NCH):
                            o = int(pack.O_c[c])
                            sc = int(pack.S_c[c])
                            if sc == 0:
                                continue
                            gl = vp.tile([P, mx // P, D2], f16, tag="V")
                            gr = vp.tile([P, mx // P, D2], f16, tag="V")
                            for g0 in range(0, sc, GCAP):
                                gn = min(GCAP, sc - g0)
                                nc.gpsimd.dma_gather(
                                    gl[:, g0 // P:(g0 + gn) // P, :],
                                    bounce2[:],
                                    gtL[:, (o + g0) // 16:
                                        (o + g0 + gn) // 16],
                                    gn, gn, D2, queue_num=qcnt[0] % cfg.NQ,
                                    single_packet=SINGLE_PACKET)
                                qcnt[0] += 1
                                nc.gpsimd.dma_gather(
                                    gr[:, g0 // P:(g0 + gn) // P, :],
                                    t_out[c * CHUNK:(c + 1) * CHUNK, :],
                                    gtR[:, (o + g0) // 16:
                                        (o + g0 + gn) // 16],
                                    gn, gn, D2, queue_num=qcnt[0] % cfg.NQ,
                                    single_packet=SINGLE_PACKET)
                                qcnt[0] += 1
                            nt_c = sc // P
                            nc.sync.dma_start(
                                boo3[:, o // P:o // P + nt_c, 0:D],
                                gl[:, :nt_c, 0:D])
                            nc.sync.dma_start(
                                boo3[:, o // P:o // P + nt_c, D:D2],
                                gr[:, :nt_c, 0:D])
                if do_ag:
                    with nc.named_scope("ag_oo"):
                        nc.gpsimd.collective_compute(
                            "AllGather", ts.bypass, replica_groups=rg,
                            ins=[bounce_oo[:].opt()], outs=[t_oo[:].opt()])

                # stage 35: [z1 | v1] = NB @ [out | outS]  (dual)
                bounce3 = dp.tile([cfg.RP, D2], f16, name="bn_zv")
                t_zv = dp.tile([NP, D2], f16, addr_space="Shared",
                               name="tb_zv")
                if do_rest:
                    with nc.named_scope("s35_NB"):
                        spmm(graphs["NBoo"], blobs["NBoo"], t_oo[:],
                             dual=True, chunk_rows=CHUNK4)
                    nc.sync.dma_start(
                        bounce3[:].rearrange("(p w) d -> p (w d)", p=P),
                        acc[:])
                if do_ag:
                    with nc.named_scope("ag_zv"):
                        nc.gpsimd.collective_compute(
                            "AllGather", ts.bypass, replica_groups=rg,
                            ins=[bounce3[:].opt()], outs=[t_zv[:].opt()])

                # stage 46: [z_pos | z_neg] = NB @ [z1 | v1]
                if do_rest:
                    with nc.named_scope("s46_NB"):
                        spmm(graphs["NB"], blobs["NB"], t_zv[:], dual=True)
                nc.sync.dma_start(
                    zpos_dev[:].rearrange("p (w d) -> p w d", d=D),
                    a3[:, :, 0:D])
                nc.sync.dma_start(
                    zneg_dev[:].rearrange("p (w d) -> p w d", d=D),
                    a3[:, :, D:D2])

    nc.compile()
    return nc


# ----------------------------------------------------------------- driver --

def _prep(cfg, x, shuf, edge_index, edge_weight, nb_index, nb_weight):
    row = edge_index[0].astype(np.int64)
    col = edge_index[1].astype(np.int64)
    ew = edge_weight.astype(np.float32)
    deg = np.zeros(cfg.N, np.float32)
    np.add.at(deg, row, ew)
    dis = np.where(deg > 0, 1.0 / np.sqrt(np.maximum(deg, 1e-30)), 0.0) \
        .astype(np.float32)
    w_norm = dis[row] * ew * dis[col]
    nrow = nb_index[0].astype(np.int64)
    ncol = nb_index[1].astype(np.int64)
    nwv = nb_weight.astype(np.float32)
    pack = ShufPack(cfg, shuf)
    gL = Graph(cfg, row, col, w_norm)
    gNB = Graph(cfg, nrow, ncol, nwv)
    gNBoo = Graph(cfg, nrow, ncol, nwv, gi_fn=pack.gi_fn)
    return gL, gNB, gNBoo, pack


def _make_in_maps(cfg, graphs, pack, x, temp):
    D, D2 = cfg.D, 2 * cfg.D
    xdev = np.zeros((cfg.NP, D2), np.float16)
    xdev[_slot(cfg, np.arange(cfg.N)), :D] = x.astype(np.float16)
    iota = np.tile(np.arange(P, dtype=np.float16), (P, 1))
    T = np.maximum(np.asarray(temp, np.float64), 0.0)
    c0, c1 = T[0], T[1] - T[0]
    c2 = (T[0] + T[2] - 2.0 * T[1]) / 4.0
    cob = np.tile(np.array([c0, c1, c2, 0.0], np.float32), (P, 1))
    in_maps = []
    for k in range(cfg.C):
        xr_k = xdev[k * cfg.RP:(k + 1) * cfg.RP, :D] \
            .reshape(P, cfg.W * D).copy()
        m = {"xtab": xdev, "xrows": xr_k, "cob": cob, "iota": iota,
             "giL": pack.idxL[k], "giR": pack.idxR[k]}
        for name, g in graphs.items():
            m[f"gi_{name}"] = g.gidx[k]
            m[f"ld_{name}"] = g.ldw[k]
            m[f"wv_{name}"] = g.wvv[k]
        in_maps.append(m)
    return in_maps


def run_pipeline(cfg, x, shuf, edge_index, edge_weight, nb_index, nb_weight,
                 temp, trace=False):
    from concourse.bass_utils import run_bass_kernel_spmd

    x = np.asarray(x, np.float32)
    gL, gNB, gNBoo, pack = _prep(
        cfg, x, np.asarray(shuf), np.asarray(edge_index),
        np.asarray(edge_weight), np.asarray(nb_index),
        np.asarray(nb_weight))
    graphs = {"L": gL, "NB": gNB, "NBoo": gNBoo}
    nc = build_program(cfg, graphs, pack)
    in_maps = _make_in_maps(cfg, graphs, pack, x, temp)
    res = run_bass_kernel_spmd(nc, in_maps, core_ids=list(range(cfg.C)),
                               trace=trace)
    outs, zps, zns = [], [], []
    for k in range(cfg.C):
        outs.append(_from_dev_rows(cfg, res.results[k]["out_dev"]
                                   .astype(np.float32)))
        zps.append(_from_dev_rows(cfg, res.results[k]["zpos_dev"]
                                  .astype(np.float32)))
        zns.append(_from_dev_rows(cfg, res.results[k]["zneg_dev"]
                                  .astype(np.float32)))
    out = (np.concatenate(outs), np.concatenate(zps), np.concatenate(zns))
    return out, res


def make_runner(nc, in_maps, n_cores):
    """Device-resident repeated-execution runner for timing (axon path)."""
    import jax
    from jax.experimental.shard_map import shard_map
    from jax.sharding import Mesh, NamedSharding, PartitionSpec

    import concourse.mybir as mybir
    from concourse import bass2jax as bj

    bj.install_neuronx_cc_hook()
    partition_name = (nc.partition_id_tensor.name
                      if nc.partition_id_tensor else None)
    in_names, out_names, out_avals, zero_outs = [], [], [], []
    for alloc in nc.m.functions[0].allocations:
        if not isinstance(alloc, mybir.MemoryLocationSet):
            continue
        name = alloc.memorylocations[0].name
        if alloc.kind == "ExternalInput":
            if name != partition_name:
                in_names.append(name)
        elif alloc.kind == "ExternalOutput":
            shape = tuple(alloc.tensor_shape)
            dtype = mybir.dt.np(alloc.dtype)
            out_names.append(name)
            out_avals.append(jax.core.ShapedArray(shape, dtype))
            zero_outs.append(np.zeros(shape, dtype))
    n_params = len(in_names)
    in_names.extend(out_names)
    if partition_name is not None:
        in_names.append(partition_name)

    def _body(*args):
        operands = list(args)
        if partition_name is not None:
            operands.append(bj.partition_id_tensor())
        outs = bj._bass_exec_p.bind(
            *operands, out_avals=tuple(out_avals),
            in_names=tuple(in_names), out_names=tuple(out_names),
            lowering_input_output_aliases=(),
            sim_require_finite=True, sim_require_nnan=True, nc=nc)
        return tuple(outs)

    devices = jax.devices()[:n_cores]
    mesh = Mesh(np.asarray(devices), ("core",))
    spec = PartitionSpec("core")
    nio = n_params + len(out_names)
    fn = jax.jit(shard_map(_body, mesh=mesh, in_specs=(spec,) * nio,
                           out_specs=(spec,) * len(out_names),
                           check_rep=False), keep_unused=True)
    concat = [np.concatenate([np.asarray(m[nm]) for m in in_maps])
              for nm in in_names[:n_params]]
    concat += [np.zeros((n_cores * z.shape[0], *z.shape[1:]), z.dtype)
               for z in zero_outs]
    sh = NamedSharding(mesh, spec)
    dev_in = [jax.device_put(a, sh) for a in concat]
    return fn, dev_in, out_names, out_avals


def timed_pipeline(cfg, x, shuf, edge_index, edge_weight, nb_index,
                   nb_weight, temp, iters=10, repeat=1, variant="full"):
    import time as _time

    import jax

    x = np.asarray(x, np.float32)
    gL, gNB, gNBoo, pack = _prep(
        cfg, x, np.asarray(shuf), np.asarray(edge_index),
        np.asarray(edge_weight), np.asarray(nb_index),
        np.asarray(nb_weight))
    graphs = {"L": gL, "NB": gNB, "NBoo": gNBoo}
    print("[timed] building program...", flush=True)
    nc = build_program(cfg, graphs, pack, repeat=repeat, variant=variant)
    print("[timed] program built", flush=True)
    in_maps = _make_in_maps(cfg, graphs, pack, x, temp)
    fn, dev_in, out_names, out_avals = make_runner(nc, in_maps, cfg.C)
    print("[timed] inputs on device, warming up...", flush=True)
    r = fn(*dev_in)
    jax.block_until_ready(r)       # warmup / compile
    print("[timed] warmup done", flush=True)
    t0 = _time.time()
    for _ in range(iters):
        r = fn(*dev_in)
    jax.block_until_ready(r)
    dt_pipe = (_time.time() - t0) / iters
    t0 = _time.time()
    for _ in range(3):
        r = fn(*dev_in)
        jax.block_until_ready(r)
    dt_sync = (_time.time() - t0) / 3
    res = {name: np.concatenate(
        [_from_dev_rows(cfg, np.asarray(r[i]).reshape(
            cfg.C, *out_avals[i].shape)[k].astype(np.float32))
         for k in range(cfg.C)])
        for i, name in enumerate(out_names)}
    out = (res["out_dev"], res["zpos_dev"], res["zneg_dev"])
    return out, dt_pipe, dt_sync


def kernel(x, shuf, edge_index, edge_weight, nb_index, nb_weight, temp):
    out, _ = run_pipeline(Cfg(), x, shuf, edge_index, edge_weight,
                          nb_index, nb_weight, temp)
    return out



# revision 6
# speedup vs baseline: 1.0169x; 1.0169x over previous
"""Trainium2 Bass kernel for nn_Bernprop2 (BernNet-style GNN propagation).

Destination-node sharding across 8 cores. Each SpMM stage: dma_gather
source rows (int16 indices into 2-rank chunks, 256B fp16 rows) -> fused
one-hot S build on DVE (single tensor_scalar: (iota==ld)*wv, fp16, 2x
perf mode) -> TensorE fp16 matmul segment-sum accumulating per 128-row
window in PSUM (fp32) -> ACT copy into an fp16 SBUF accumulator.
Inter-stage tables are exchanged with ncfw AllGather (fp16) into
internal Shared DRAM.

Stage graph (5 SpMMs instead of reference's 6):
  s1: sp1 = Anorm @ x         (gather xtab)        -> lx = x - sp1
  s2: sp2 = Anorm @ lx        (gather t_lx)        -> out = c0 x + c1 lx + c2 (lx - sp2)
  s3: z1 = NB @ out           (gather t_out)       -> acc left halves
  s5: v1 = NS @ out           (gather t_out)       -> acc right halves
  s46: [z_pos | z_neg] = NB @ [z1 | v1]  (gather t_zv, dual-width rows)

Tables live in a permuted "device layout": node n -> slot
k*RP + p*W + w  (k=n//R, r=n%R, w=r//P, p=r%P) so every table write is one
contiguous DMA and gather indices within a 2-rank chunk fit in int16.
Table rows are 128 fp16 elems (256B - the dma_gather minimum); single-width
stages use the lower 64, the dual stage uses all 128.
"""

import sys

if "/opt/trn_rl_repo" not in sys.path:
    sys.path.insert(0, "/opt/trn_rl_repo")

import numpy as np

P = 128   # partitions / window rows / tile edges
GCAP = 2048   # max indices per dma_gather call; finer beats coarser on
              # HW (queue interleave + earlier first matmul per cell),
              # but 1280 and 1024 hang the device intermittently —
              # 2048 is the fastest config with zero observed hangs.
SCRATCH = 32768   # SWDGE ring bytes (2048 descriptors)
SINGLE_PACKET = False  # dma_gather packetization mode
BUFS = 5      # buffer depth for blob/V/S pools (5 > 4 > 3 > 2 on HW)
PP_BUFS = 8   # PSUM tile pool depth (window accumulators)


class Cfg:
    def __init__(self, N=100000, E=1250000, D=64, C=8, block_w=8,
                 n_queues=4):
        self.N, self.E, self.D, self.C = N, E, D, C
        self.NQ = n_queues
        assert N % C == 0
        self.R = N // C                     # rows per core
        self.W = -(-self.R // P)            # windows per core
        self.RP = self.W * P                # padded rows per core
        self.NP = self.C * self.RP          # padded table rows
        self.CHUNK = 2 * self.RP            # rows per gather chunk (2 ranks)
        assert self.CHUNK <= 32767
        self.NCH = self.C // 2              # number of chunks
        self.BLOCK_W = block_w              # windows per block
        self.NBLK = -(-self.W // self.BLOCK_W)


def _slot(cfg, n):
    """Global device-table slot for node id array n."""
    k = n // cfg.R
    r = n - k * cfg.R
    return k * cfg.RP + (r % P) * cfg.W + (r // P)


def _chunk_idx(cfg, n):
    """(chunk id, int16 index within chunk) for source node array n."""
    k = n // cfg.R
    r = n - k * cfg.R
    return (k >> 1), (k & 1) * cfg.RP + (r % P) * cfg.W + (r // P)


def _from_dev_rows(cfg, a):
    """[P, W*D] per-core device rows -> [R, D]."""
    full = a.reshape(P, cfg.W, cfg.D).transpose(1, 0, 2).reshape(cfg.RP,
                                                                 cfg.D)
    return full[: cfg.R]


class ShufPack:
    """Per-core padded chunk-sorted layout for the [out | out[shuf]] table.

    Core k's section (RP4 rows): its RP slots permuted so slots whose
    shuf-target lives in chunk c form the contiguous segment
    [O_c, O_c + cnt_kc), padded (idx 0) up to the uniform S_c.
    """

    def __init__(self, cfg, shuf):
        C, R, W, RP, CHUNK = cfg.C, cfg.R, cfg.W, cfg.RP, cfg.CHUNK
        s_all = np.arange(RP)
        p, w = s_all // W, s_all % W
        r = w * P + p                    # local row for each slot
        real = r < R
        per_core = []
        cnts = np.zeros((C, cfg.NCH), np.int64)
        shuf = np.asarray(shuf, np.int64)
        for k in range(C):
            n = np.minimum(k * R + r, cfg.N - 1)
            st = np.where(real, _slot(cfg, shuf[n]), 0)
            c = st // CHUNK
            order = np.argsort(c, kind="stable")
            per_core.append((order, st[order], c[order]))
            cnts[k] = np.bincount(c, minlength=cfg.NCH)
        S_c = ((cnts.max(axis=0) + 127) // 128) * 128
        self.O_c = np.concatenate([[0], np.cumsum(S_c)]).astype(np.int64)
        self.S_c = S_c
        self.RP4 = int(self.O_c[-1])
        self.CHUNK4 = 2 * self.RP4
        assert self.CHUNK4 <= 32767
        self.posmap = np.zeros((C, RP), np.int64)
        self.idxL, self.idxR = [], []
        for k in range(C):
            order, st_o, c_o = per_core[k]
            iL = np.zeros(self.RP4, np.int16)
            iR = np.zeros(self.RP4, np.int16)
            for c in range(cfg.NCH):
                seg = c_o == c
                cnt = int(seg.sum())
                sl = order[seg]
                o = self.O_c[c]
                iL[o:o + cnt] = sl
                iR[o:o + cnt] = st_o[seg] - c * CHUNK
                self.posmap[k, sl] = o + np.arange(cnt)
            self.idxL.append(iL.reshape(-1, 16).T.copy())
            self.idxR.append(iR.reshape(-1, 16).T.copy())

    def gi_fn(self, cfg, col):
        """(chunk, int16 idx) of col nodes in the padded t_oo layout."""
        st = _slot(cfg, col)
        k = st // cfg.RP
        pos = self.posmap[k, st % cfg.RP]
        return (k >> 1), (k & 1) * self.RP4 + pos


class Graph:
    """Shared schedule + per-core blobs for one edge list."""

    def __init__(self, cfg, row, col, wv, gi_fn=None):
        C, R, W, NCH, BW = cfg.C, cfg.R, cfg.W, cfg.NCH, cfg.BLOCK_W
        per_core = []
        counts = np.zeros((C, NCH, W), np.int64)
        for k in range(C):
            m = (row >= k * R) & (row < (k + 1) * R)
            r = row[m] - k * R
            cc, gi = (gi_fn or _chunk_idx)(cfg, col[m])
            win, ld = r // P, r % P
            order = np.lexsort((gi, ld, win, cc, win // BW))
            per_core.append((cc[order], win[order], ld[order], gi[order],
                             wv[m][order]))
            np.add.at(counts[k], (cc[order], win[order]), 1)
        maxc = counts.max(axis=0)                      # [NCH, W]
        ntile = -(-maxc // P)                          # tiles per (c, w) cell
        ntile[0] = np.maximum(ntile[0], 1)             # c0 owns start=True

        # Schedule: blocks -> cells (c, list of (w, ntiles)) in stream order.
        self.blocks = []
        tot = 0
        for b in range(cfg.NBLK):
            ws = range(b * BW, min((b + 1) * BW, W))
            cells = []
            for c in range(NCH):
                wt = [(w, int(ntile[c, w])) for w in ws if ntile[c, w] > 0]
                n = sum(t for _, t in wt) * P
                cells.append((c, tot, n, wt))
                tot += n
            self.blocks.append((list(ws), cells))
        self.total = tot
        self.max_cell = max((n for (_, cells) in self.blocks
                             for (_, _, n, _) in cells), default=0)
        self.max_blk = max((sum(n for (_, _, n, _) in cells)
                            for _, cells in self.blocks), default=0)

        # Per-core blobs in schedule layout.  Pad slots: gi=0 (gathers row 0,
        # harmless), wv=0 (zero weight), ld=0.
        self.gidx, self.ldw, self.wvv = [], [], []
        for k in range(C):
            cc, win, ld, gi, wv_ = per_core[k]
            g16 = np.zeros(tot, np.int16)
            ldf = np.zeros(tot, np.float32)
            wvf = np.zeros(tot, np.float32)
            starts = {}
            for ws_, cells in self.blocks:
                for (c, off, n, wt) in cells:
                    o = off
                    for (w, t) in wt:
                        starts[(c, w)] = o
                        o += t * P
            keys = cc * W + win
            uk, first, cnt = np.unique(keys, return_index=True,
                                       return_counts=True)
            for u, f, n_ in zip(uk, first, cnt):
                c, w = int(u) // W, int(u) % W
                o = starts[(c, w)]
                g16[o:o + n_] = gi[f:f + n_]
                ldf[o:o + n_] = ld[f:f + n_]
                wvf[o:o + n_] = wv_[f:f + n_]
            self.gidx.append(g16.reshape(-1, 16).T.copy())   # [16, tot/16]
            # 4x-expanded fp16 blobs: keeps DVE 2x mode (packed last dim)
            self.ldw.append(np.repeat(ldf.reshape(-1, P).T, 4, axis=1)
                            .astype(np.float16))             # [P, tot/P*4]
            self.wvv.append(np.repeat(wvf.reshape(-1, P).T, 4, axis=1)
                            .astype(np.float16))


# ---------------------------------------------------------------- builder --

def build_program(cfg, graphs, pack, repeat=1, variant="full"):
    """graphs = dict(L=Graph, NB=Graph, NBoo=Graph). Returns compiled nc."""
    import concourse.bacc as bacc
    import concourse.mybir as mybir
    import concourse.tile as tile

    D, W, NP, CHUNK = cfg.D, cfg.W, cfg.NP, cfg.CHUNK
    RP4, CHUNK4 = pack.RP4, pack.CHUNK4
    D2 = 2 * D
    f16 = mybir.dt.float16
    f32 = mybir.dt.float32
    ts = mybir.AluOpType
    nc = bacc.Bacc("TRN2", target_bir_lowering=False, debug=False,
                   num_devices=cfg.C, num_swdge_queues=cfg.NQ,
                   dynamic_dma_scratch_size=SCRATCH)

    # I/O ------------------------------------------------------------------
    xtab = nc.dram_tensor("xtab", [NP, D2], f16, kind="ExternalInput")
    xrows = nc.dram_tensor("xrows", [P, W * D], f16, kind="ExternalInput")
    cob_in = nc.dram_tensor("cob", [P, 4], f32, kind="ExternalInput")
    iota_in = nc.dram_tensor("iota", [P, P], f16, kind="ExternalInput")
    blobs = {}
    for name, g in graphs.items():
        blobs[name] = dict(
            gi=nc.dram_tensor(f"gi_{name}", [16, g.total // 16],
                              mybir.dt.int16, kind="ExternalInput"),
            ld=nc.dram_tensor(f"ld_{name}", [P, g.total // P * 4], f16,
                              kind="ExternalInput"),
            wv=nc.dram_tensor(f"wv_{name}", [P, g.total // P * 4], f16,
                              kind="ExternalInput"),
        )
    giL_in = nc.dram_tensor("giL", [16, RP4 // 16], mybir.dt.int16,
                            kind="ExternalInput")
    giR_in = nc.dram_tensor("giR", [16, RP4 // 16], mybir.dt.int16,
                            kind="ExternalInput")
    out_dev = nc.dram_tensor("out_dev", [P, W * D], f16,
                             kind="ExternalOutput")
    zpos_dev = nc.dram_tensor("zpos_dev", [P, W * D], f16,
                              kind="ExternalOutput")
    zneg_dev = nc.dram_tensor("zneg_dev", [P, W * D], f16,
                              kind="ExternalOutput")

    rg = [list(range(cfg.C))]
    mx = max(max(g.max_cell for g in graphs.values()),
             int(pack.S_c.max()))
    mxb = max(g.max_blk for g in graphs.values())

    with tile.TileContext(nc) as tc:
        with (
            tc.tile_pool(name="const", bufs=1) as constp,
            tc.tile_pool(name="acc", bufs=2 if repeat > 1 else 1) as accp,
            tc.tile_pool(name="blob", bufs=BUFS) as blobp,
            tc.tile_pool(name="vg", bufs=BUFS) as vp,
            tc.tile_pool(name="sm", bufs=BUFS) as sp,
            tc.tile_pool(name="ps", bufs=max(PP_BUFS, cfg.BLOCK_W),
                         space="PSUM") as pp,
            tc.tile_pool(name="dram", bufs=1, space="DRAM") as dp,
        ):
            iota_t = constp.tile([P, P], f16, name="iota_t")
            nc.sync.dma_start(iota_t[:], iota_in[:])
            xr = constp.tile([P, W * D], f16, name="xr")
            nc.sync.dma_start(xr[:], xrows[:])
            lxr = constp.tile([P, W * D], f16, name="lxr")
            outr = constp.tile([P, W * D], f16, name="outr")
            cob = constp.tile([P, 4], f32, name="cob")
            nc.sync.dma_start(cob[:], cob_in[:])
            gtL = constp.tile([P, RP4 // 16], mybir.dt.int16, name="gtL")
            gtR = constp.tile([P, RP4 // 16], mybir.dt.int16, name="gtR")
            for r in range(8):
                nc.sync.dma_start(gtL[16 * r:16 * (r + 1), :], giL_in[:])
                nc.sync.dma_start(gtR[16 * r:16 * (r + 1), :], giR_in[:])
            xr3 = xr[:].rearrange("p (w d) -> p w d", d=D)
            lxr3 = lxr[:].rearrange("p (w d) -> p w d", d=D)
            outr3 = outr[:].rearrange("p (w d) -> p w d", d=D)

            qcnt = [0]
            cur = {}  # per-rep acc tile + views (rotated via accp bufs)

            def spmm(g, blob, table_ap, dual=False, half=0,
                     chunk_rows=CHUNK):
                """One SpMM stage.

                dual: V/psum are 128 wide, result -> full acc rows.
                else: 64 wide, result -> acc half `half` (0=lo, 1=hi).
                """
                wdt = D2 if dual else D
                for ws, cells in g.blocks:
                    blk_n = sum(n for (_, _, n, _) in cells)
                    if blk_n == 0:
                        continue
                    b_off = cells[0][1]
                    gt = blobp.tile([P, mxb // 16], mybir.dt.int16, tag="gt")
                    lt = blobp.tile([P, mxb // P * 4], f16, tag="lt")
                    wt_ = blobp.tile([P, mxb // P * 4], f16, tag="wt")
                    for r in range(8):
                        nc.sync.dma_start(
                            gt[16 * r:16 * (r + 1), : blk_n // 16],
                            blob["gi"][:, b_off // 16:(b_off + blk_n) // 16])
                    nc.sync.dma_start(
                        lt[:, : blk_n // P * 4],
                        blob["ld"][:, b_off // P * 4:(b_off + blk_n) // P * 4])
                    nc.sync.dma_start(
                        wt_[:, : blk_n // P * 4],
                        blob["wv"][:, b_off // P * 4:(b_off + blk_n) // P * 4])
                    ptiles = {w: pp.tile([P, wdt], f32, tag="psw",
                                         name=f"ps_{w}") for w in ws}
                    touched = set()
                    for (c, off, n, wtl) in cells:
                        if n == 0:
                            continue
                        nt = n // P
                        lo = off - b_off
                        V = vp.tile([P, mx // P, D2], f16, tag="V")
                        for g0 in range(0, n, GCAP):
                            gn = min(GCAP, n - g0)
                            nc.gpsimd.dma_gather(
                                V[:, g0 // P:(g0 + gn) // P, :],
                                table_ap[c * chunk_rows:
                                         (c + 1) * chunk_rows, :],
                                gt[:, (lo + g0) // 16:(lo + g0 + gn) // 16],
                                gn, gn, D2, queue_num=qcnt[0] % cfg.NQ,
                                single_packet=SINGLE_PACKET)
                            qcnt[0] += 1
                        S = sp.tile([P, (mx // P) * P], f16, tag="S")
                        s4 = S[:].rearrange("p (t a b) -> p t a b",
                                            a=32, b=4)[:, :nt]
                        iob = iota_t[:].rearrange("p (a b) -> p a b", b=4) \
                            .unsqueeze(1).to_broadcast([P, nt, 32, 4])
                        c4 = lo // P * 4
                        ld4 = lt[:, c4:c4 + nt * 4] \
                            .rearrange("p (t b) -> p t b", b=4) \
                            .unsqueeze(2).to_broadcast([P, nt, 32, 4])
                        nc.vector.tensor_tensor(s4, iob, ld4,
                                                op=ts.is_equal)
                        wv4 = wt_[:, c4:c4 + nt * 4] \
                            .rearrange("p (t b) -> p t b", b=4)
                        if dual:
                            wb = wv4.unsqueeze(2).to_broadcast([P, nt, 32, 4])
                            nc.vector.tensor_tensor(s4, s4, wb, op=ts.mult)
                        else:
                            v4 = V[:, :nt, 0:D].rearrange(
                                "p t (a b) -> p t a b", b=4)
                            wb = wv4.unsqueeze(2).to_broadcast([P, nt, 16, 4])
                            nc.vector.tensor_tensor(v4, v4, wb, op=ts.mult)
                        ti = 0
                        for (w, t) in wtl:
                            for j in range(t):
                                last = (c == max(
                                    cx for (cx, _, nx, wl) in cells
                                    if nx and any(wx == w for wx, _ in wl))
                                    and j == t - 1)
                                nc.tensor.matmul(
                                    ptiles[w][:],
                                    S[:, (ti + j) * P:(ti + j + 1) * P],
                                    V[:, ti + j, 0:wdt],
                                    start=(w not in touched),
                                    stop=last)
                                touched.add(w)
                            ti += t
                    for w in ws:
                        if dual:
                            dst = cur["a3"][:, w, :]
                        else:
                            dst = cur["a3"][:, w, half * D:half * D + D]
                        nc.any.tensor_copy(dst, ptiles[w][:])

            do_s1, do_s2, do_mk, do_s35, do_s46, do_ag = {
                "empty":  (0, 0, 0, 0, 0, 0),
                "s1":     (1, 0, 0, 0, 0, 0),
                "s12":    (1, 1, 0, 0, 0, 0),
                "s12m":   (1, 1, 1, 0, 0, 0),
                "s12m35": (1, 1, 1, 1, 0, 0),
                "noag":   (1, 1, 1, 1, 1, 0),
                "full":   (1, 1, 1, 1, 1, 1),
            }[variant]
            for _rep in range(repeat):
                # per-rep accumulator (bufs=2 -> reps double-buffer)
                acc = accp.tile([P, W * D2], f16, name="acc_t", tag="acc_t")
                a3 = acc[:].rearrange("p (w d) -> p w d", d=D2)
                acc_lo = a3[:, :, 0:D]           # [P, W, D] strided views
                acc_hi = a3[:, :, D:D2]
                cur["a3"] = a3
                # upper halves of acc rows must be zero for s1/s2 bounces
                nc.vector.memset(acc_hi, 0.0)
                if not do_s1:
                    nc.vector.memset(acc_lo, 0.0)
                if not do_s2:
                    nc.vector.memset(outr[:], 0.0)

                # stage 1: sp1 = Anorm @ x ; lx = x - sp1
                bounce1 = dp.tile([cfg.RP, D2], f16, name="bn_lx")
                t_lx = dp.tile([NP, D2], f16, addr_space="Shared",
                               name="tb_lx")
                if do_s1:
                    with nc.named_scope("s1_L"):
                        spmm(graphs["L"], blobs["L"], xtab[:])
                        nc.vector.tensor_tensor(acc_lo, xr3, acc_lo,
                                                op=ts.subtract)
                        nc.vector.tensor_copy(lxr3, acc_lo)
                    nc.sync.dma_start(
                        bounce1[:].rearrange("(p w) d -> p (w d)", p=P),
                        acc[:])
                if do_ag:
                    with nc.named_scope("ag_lx"):
                        nc.gpsimd.collective_compute(
                            "AllGather", ts.bypass, replica_groups=rg,
                            ins=[bounce1[:].opt()], outs=[t_lx[:].opt()])

                # stage 2: sp2 = Anorm @ lx ; out = c0 x + c1 lx + c2 LLx
                bounce2 = dp.tile([cfg.RP, D2], f16, name="bn_out")
                t_out = dp.tile([NP, D2], f16, addr_space="Shared",
                                name="tb_out")
                if do_s2:
                    with nc.named_scope("s2_L"):
                        spmm(graphs["L"], blobs["L"], t_lx[:])
                        # acc_lo = sp2 -> LLx = lx - sp2
                        nc.vector.tensor_tensor(acc_lo, lxr3, acc_lo,
                                                op=ts.subtract)
                        nc.vector.tensor_scalar(acc_lo, acc_lo, cob[:, 2:3],
                                                None, op0=ts.mult)
                        nc.vector.tensor_scalar(outr[:], xr[:], cob[:, 0:1],
                                                None, op0=ts.mult)
                        nc.vector.tensor_tensor(outr3, outr3, acc_lo,
                                                op=ts.add)
                        nc.vector.tensor_scalar(lxr[:], lxr[:], cob[:, 1:2],
                                                None, op0=ts.mult)
                        nc.vector.tensor_tensor(outr[:], outr[:], lxr[:],
                                                op=ts.add)
                        nc.vector.tensor_copy(acc_lo, outr3)
                nc.sync.dma_start(out_dev[:], outr[:])
                if do_s2:
                    nc.sync.dma_start(
                        bounce2[:].rearrange("(p w) d -> p (w d)", p=P),
                        acc[:])
                if do_ag:
                    with nc.named_scope("ag_out"):
                        nc.gpsimd.collective_compute(
                            "AllGather", ts.bypass, replica_groups=rg,
                            ins=[bounce2[:].opt()], outs=[t_out[:].opt()])

                # build t_oo: rows [out_i | out_shuf_i] in the padded
                # chunk-sorted per-core layout (pack).  Left halves gather
                # from the LOCAL bounce2 (no AG dependency); right halves
                # from t_out after the AG.
                bounce_oo = dp.tile([RP4, D2], f16, name="bn_oo")
                t_oo = dp.tile([cfg.C * RP4, D2], f16, addr_space="Shared",
                               name="tb_oo")
                boo3 = bounce_oo[:].rearrange("(t p) d -> p t d", p=P)
                if do_mk:
                    with nc.named_scope("mk_oo"):
                        for c in range(cfg.NCH):
                            o = int(pack.O_c[c])
                            sc = int(pack.S_c[c])
                            if sc == 0:
                                continue
                            gl = vp.tile([P, mx // P, D2], f16, tag="V")
                            gr = vp.tile([P, mx // P, D2], f16, tag="V")
                            for g0 in range(0, sc, GCAP):
                                gn = min(GCAP, sc - g0)
                                nc.gpsimd.dma_gather(
                                    gl[:, g0 // P:(g0 + gn) // P, :],
                                    bounce2[:],
                                    gtL[:, (o + g0) // 16:
                                        (o + g0 + gn) // 16],
                                    gn, gn, D2, queue_num=qcnt[0] % cfg.NQ,
                                    single_packet=SINGLE_PACKET)
                                qcnt[0] += 1
                                nc.gpsimd.dma_gather(
                                    gr[:, g0 // P:(g0 + gn) // P, :],
                                    t_out[c * CHUNK:(c + 1) * CHUNK, :],
                                    gtR[:, (o + g0) // 16:
                                        (o + g0 + gn) // 16],
                                    gn, gn, D2, queue_num=qcnt[0] % cfg.NQ,
                                    single_packet=SINGLE_PACKET)
                                qcnt[0] += 1
                            nt_c = sc // P
                            nc.sync.dma_start(
                                boo3[:, o // P:o // P + nt_c, 0:D],
                                gl[:, :nt_c, 0:D])
                            nc.sync.dma_start(
                                boo3[:, o // P:o // P + nt_c, D:D2],
                                gr[:, :nt_c, 0:D])
                if do_ag:
                    with nc.named_scope("ag_oo"):
                        nc.gpsimd.collective_compute(
                            "AllGather", ts.bypass, replica_groups=rg,
                            ins=[bounce_oo[:].opt()], outs=[t_oo[:].opt()])

                # stage 35: [z1 | v1] = NB @ [out | outS]  (dual)
                bounce3 = dp.tile([cfg.RP, D2], f16, name="bn_zv")
                t_zv = dp.tile([NP, D2], f16, addr_space="Shared",
                               name="tb_zv")
                if do_s35:
                    with nc.named_scope("s35_NB"):
                        spmm(graphs["NBoo"], blobs["NBoo"], t_oo[:],
                             dual=True, chunk_rows=CHUNK4)
                    nc.sync.dma_start(
                        bounce3[:].rearrange("(p w) d -> p (w d)", p=P),
                        acc[:])
                if do_ag:
                    with nc.named_scope("ag_zv"):
                        nc.gpsimd.collective_compute(
                            "AllGather", ts.bypass, replica_groups=rg,
                            ins=[bounce3[:].opt()], outs=[t_zv[:].opt()])

                # stage 46: [z_pos | z_neg] = NB @ [z1 | v1]
                if do_s46:
                    with nc.named_scope("s46_NB"):
                        spmm(graphs["NB"], blobs["NB"], t_zv[:], dual=True)
                nc.sync.dma_start(
                    zpos_dev[:].rearrange("p (w d) -> p w d", d=D),
                    a3[:, :, 0:D])
                nc.sync.dma_start(
                    zneg_dev[:].rearrange("p (w d) -> p w d", d=D),
                    a3[:, :, D:D2])

    nc.compile()
    return nc


# ----------------------------------------------------------------- driver --

def _prep(cfg, x, shuf, edge_index, edge_weight, nb_index, nb_weight):
    row = edge_index[0].astype(np.int64)
    col = edge_index[1].astype(np.int64)
    ew = edge_weight.astype(np.float32)
    deg = np.zeros(cfg.N, np.float32)
    np.add.at(deg, row, ew)
    dis = np.where(deg > 0, 1.0 / np.sqrt(np.maximum(deg, 1e-30)), 0.0) \
        .astype(np.float32)
    w_norm = dis[row] * ew * dis[col]
    nrow = nb_index[0].astype(np.int64)
    ncol = nb_index[1].astype(np.int64)
    nwv = nb_weight.astype(np.float32)
    pack = ShufPack(cfg, shuf)
    gL = Graph(cfg, row, col, w_norm)
    gNB = Graph(cfg, nrow, ncol, nwv)
    gNBoo = Graph(cfg, nrow, ncol, nwv, gi_fn=pack.gi_fn)
    return gL, gNB, gNBoo, pack


def _make_in_maps(cfg, graphs, pack, x, temp):
    D, D2 = cfg.D, 2 * cfg.D
    xdev = np.zeros((cfg.NP, D2), np.float16)
    xdev[_slot(cfg, np.arange(cfg.N)), :D] = x.astype(np.float16)
    iota = np.tile(np.arange(P, dtype=np.float16), (P, 1))
    T = np.maximum(np.asarray(temp, np.float64), 0.0)
    c0, c1 = T[0], T[1] - T[0]
    c2 = (T[0] + T[2] - 2.0 * T[1]) / 4.0
    cob = np.tile(np.array([c0, c1, c2, 0.0], np.float32), (P, 1))
    in_maps = []
    for k in range(cfg.C):
        xr_k = xdev[k * cfg.RP:(k + 1) * cfg.RP, :D] \
            .reshape(P, cfg.W * D).copy()
        m = {"xtab": xdev, "xrows": xr_k, "cob": cob, "iota": iota,
             "giL": pack.idxL[k], "giR": pack.idxR[k]}
        for name, g in graphs.items():
            m[f"gi_{name}"] = g.gidx[k]
            m[f"ld_{name}"] = g.ldw[k]
            m[f"wv_{name}"] = g.wvv[k]
        in_maps.append(m)
    return in_maps


def run_pipeline(cfg, x, shuf, edge_index, edge_weight, nb_index, nb_weight,
                 temp, trace=False):
    from concourse.bass_utils import run_bass_kernel_spmd

    x = np.asarray(x, np.float32)
    gL, gNB, gNBoo, pack = _prep(
        cfg, x, np.asarray(shuf), np.asarray(edge_index),
        np.asarray(edge_weight), np.asarray(nb_index),
        np.asarray(nb_weight))
    graphs = {"L": gL, "NB": gNB, "NBoo": gNBoo}
    nc = build_program(cfg, graphs, pack)
    in_maps = _make_in_maps(cfg, graphs, pack, x, temp)
    res = run_bass_kernel_spmd(nc, in_maps, core_ids=list(range(cfg.C)),
                               trace=trace)
    outs, zps, zns = [], [], []
    for k in range(cfg.C):
        outs.append(_from_dev_rows(cfg, res.results[k]["out_dev"]
                                   .astype(np.float32)))
        zps.append(_from_dev_rows(cfg, res.results[k]["zpos_dev"]
                                  .astype(np.float32)))
        zns.append(_from_dev_rows(cfg, res.results[k]["zneg_dev"]
                                  .astype(np.float32)))
    out = (np.concatenate(outs), np.concatenate(zps), np.concatenate(zns))
    return out, res


def make_runner(nc, in_maps, n_cores):
    """Device-resident repeated-execution runner for timing (axon path)."""
    import jax
    from jax.experimental.shard_map import shard_map
    from jax.sharding import Mesh, NamedSharding, PartitionSpec

    import concourse.mybir as mybir
    from concourse import bass2jax as bj

    bj.install_neuronx_cc_hook()
    partition_name = (nc.partition_id_tensor.name
                      if nc.partition_id_tensor else None)
    in_names, out_names, out_avals, zero_outs = [], [], [], []
    for alloc in nc.m.functions[0].allocations:
        if not isinstance(alloc, mybir.MemoryLocationSet):
            continue
        name = alloc.memorylocations[0].name
        if alloc.kind == "ExternalInput":
            if name != partition_name:
                in_names.append(name)
        elif alloc.kind == "ExternalOutput":
            shape = tuple(alloc.tensor_shape)
            dtype = mybir.dt.np(alloc.dtype)
            out_names.append(name)
            out_avals.append(jax.core.ShapedArray(shape, dtype))
            zero_outs.append(np.zeros(shape, dtype))
    n_params = len(in_names)
    in_names.extend(out_names)
    if partition_name is not None:
        in_names.append(partition_name)

    def _body(*args):
        operands = list(args)
        if partition_name is not None:
            operands.append(bj.partition_id_tensor())
        outs = bj._bass_exec_p.bind(
            *operands, out_avals=tuple(out_avals),
            in_names=tuple(in_names), out_names=tuple(out_names),
            lowering_input_output_aliases=(),
            sim_require_finite=True, sim_require_nnan=True, nc=nc)
        return tuple(outs)

    devices = jax.devices()[:n_cores]
    mesh = Mesh(np.asarray(devices), ("core",))
    spec = PartitionSpec("core")
    nio = n_params + len(out_names)
    fn = jax.jit(shard_map(_body, mesh=mesh, in_specs=(spec,) * nio,
                           out_specs=(spec,) * len(out_names),
                           check_rep=False), keep_unused=True)
    concat = [np.concatenate([np.asarray(m[nm]) for m in in_maps])
              for nm in in_names[:n_params]]
    concat += [np.zeros((n_cores * z.shape[0], *z.shape[1:]), z.dtype)
               for z in zero_outs]
    sh = NamedSharding(mesh, spec)
    dev_in = [jax.device_put(a, sh) for a in concat]
    return fn, dev_in, out_names, out_avals


def timed_pipeline(cfg, x, shuf, edge_index, edge_weight, nb_index,
                   nb_weight, temp, iters=10, repeat=1, variant="full"):
    import time as _time

    import jax

    x = np.asarray(x, np.float32)
    gL, gNB, gNBoo, pack = _prep(
        cfg, x, np.asarray(shuf), np.asarray(edge_index),
        np.asarray(edge_weight), np.asarray(nb_index),
        np.asarray(nb_weight))
    graphs = {"L": gL, "NB": gNB, "NBoo": gNBoo}
    print("[timed] building program...", flush=True)
    nc = build_program(cfg, graphs, pack, repeat=repeat, variant=variant)
    print("[timed] program built", flush=True)
    in_maps = _make_in_maps(cfg, graphs, pack, x, temp)
    fn, dev_in, out_names, out_avals = make_runner(nc, in_maps, cfg.C)
    print("[timed] inputs on device, warming up...", flush=True)
    r = fn(*dev_in)
    jax.block_until_ready(r)       # warmup / compile
    print("[timed] warmup done", flush=True)
    t0 = _time.time()
    for _ in range(iters):
        r = fn(*dev_in)
    jax.block_until_ready(r)
    dt_pipe = (_time.time() - t0) / iters
    t0 = _time.time()
    for _ in range(3):
        r = fn(*dev_in)
        jax.block_until_ready(r)
    dt_sync = (_time.time() - t0) / 3
    res = {name: np.concatenate(
        [_from_dev_rows(cfg, np.asarray(r[i]).reshape(
            cfg.C, *out_avals[i].shape)[k].astype(np.float32))
         for k in range(cfg.C)])
        for i, name in enumerate(out_names)}
    out = (res["out_dev"], res["zpos_dev"], res["zneg_dev"])
    return out, dt_pipe, dt_sync


def kernel(x, shuf, edge_index, edge_weight, nb_index, nb_weight, temp):
    out, _ = run_pipeline(Cfg(), x, shuf, edge_index, edge_weight,
                          nb_index, nb_weight, temp)
    return out

